# revision 50
# baseline (speedup 1.0000x reference)
"""Multi-head self-attention (B=2, T=2048, C=1024, H=16) on 8 trn2 cores.

Sharding: core c -> batch b = c//4, heads 4*(c%4) .. 4*(c%4)+3.
Each core: QKV projection for its 4 heads, causal attention in S^T layout
(keys on partitions), partial output projection over its heads' rows of Wo.
Host sums the 4 partials per batch element and adds bo.

Matmul inputs are bf16 (PSUM accumulates fp32), except the q/k projections
for t >= 512, which run in fp8e4m3 with DoubleRow (2x PE rate; Wqk
prescaled x32 host-side, folded back via the exp scale).  fp8 there only
perturbs logits, which softmax averages away for rows with many keys; the
first 512 rows (few-key softmax) and the whole v path stay bf16.

The schedule keeps the PE continuously busy: inputs arrive as a few large
strided DMAs ordered so the first projection group starts ~4us in,
projections for slab s+1 / the other head-pair / output-projection pieces
are interleaved as "filler" between software-pipelined attention chunks
(S-matmul of chunk c+1 issues before PV of chunk c, so the PE never waits
on the Act engine's exp), the softmax division runs per slab with its
PSUM readers front-loaded, and the output projection for slab s overlaps
pair-1's slab s+1 attention.

HW-validated pitfalls baked in: reciprocal_approx_fast needs a partition-0
SBUF input; two-SBUF-input DVE ops need matching partition bases; gpsimd
cannot touch PSUM (CoreSim accepts all of these, hardware does not).
"""
import sys

sys.path.insert(0, "/opt/trn_rl_repo")

import numpy as np

B, T, C, H = 2, 2048, 1024, 16
HD = C // H            # 64
NCORES = 8
HPC = H // (NCORES // B)   # heads per core = 4
QB = 128               # q block (columns of S^T)
KB = 128               # k chunk (partitions of S^T)
NJ = T // KB           # 16
NI = T // QB           # 16
SLAB = 512             # q columns processed per attention pass
NSLAB = T // SLAB      # 4
BPS = SLAB // QB       # q blocks per slab = 4
CI = C // 128          # 8 contraction chunks for projections
SCALE = HD ** -0.5
# fp8e4m3 + DoubleRow (2x PE rate) for the q/k projections only (Wqkv
# prescaled x32, folded back via the exp scale).  Quantization there only
# perturbs logits, which softmax dampens.  The v path (projection, PV) stays
# bf16 — v errors hit the output directly and early causal rows have no
# softmax averaging to wash them out — so x is staged twice: fp8 for q/k,
# bf16 for v.
FP8_PROJ = True
FP8_PV = False

_cache = {}


def _build_plan(mask_bool: np.ndarray):
    """mask_bool: [T, T] (q, k). Returns per (j, i) block types and tiles.

    type 0 = all valid (no mask work), 1 = all masked (skip), 2 = mixed.
    Tiles are stored transposed to match S^T ([k_local, q_local])."""
    btype = np.zeros((NJ, NI), dtype=np.int32)
    tidx = np.full((NJ, NI), -1, dtype=np.int32)
    tiles = []
    tile_map = {}
    for j in range(NJ):
        for i in range(NI):
            sub = mask_bool[i * QB:(i + 1) * QB, j * KB:(j + 1) * KB]
            if sub.all():
                btype[j, i] = 0
            elif not sub.any():
                btype[j, i] = 1
            else:
                btype[j, i] = 2
                key = sub.tobytes()
                if key not in tile_map:
                    tile_map[key] = len(tiles)
                    tiles.append(sub.T.astype(np.float32))
                tidx[j, i] = tile_map[key]
    if not tiles:
        tiles.append(np.ones((KB, QB), dtype=np.float32))
    return btype, tidx, np.stack(tiles)


def _chunk_list(btype, s):
    """Live (j, i0, i1) chunk runs for slab s."""
    i_lo, i_hi = s * BPS, (s + 1) * BPS
    chunks = []
    for j in range(NJ):
        live = [i for i in range(i_lo, i_hi) if btype[j, i] != 1]
        if live:
            chunks.append((j, min(live), max(live)))
    return chunks


def _build_program(btype, tidx, n_tiles, apply_qk_bias, apply_v_bias):
    import concourse.tile as tile
    import concourse.mybir as mybir
    from concourse import bacc

    F32 = mybir.dt.float32
    BF16 = mybir.dt.bfloat16
    F8 = mybir.dt.float8e4
    XDT = F8 if FP8_PROJ else BF16
    PDT = F8 if FP8_PV else BF16
    DR = mybir.MatmulPerfMode.DoubleRow
    ESCALE = SCALE / 1024.0 if FP8_PROJ else SCALE
    # shift logits so exp() stays well inside fp8e4m3 range; the softmax
    # ratio is shift-invariant (numerator and denominator share e^-3)
    EBIAS = -3.0 if FP8_PV else 0.0
    AF = mybir.ActivationFunctionType
    MULT = mybir.AluOpType.mult

    nc = bacc.Bacc("TRN2", target_bir_lowering=False, debug=False)
    xt_d = nc.dram_tensor("xt", [C, T], XDT, kind="ExternalInput").ap()
    if FP8_PROJ:
        xtb_d = nc.dram_tensor("xtb", [C, T], BF16,
                               kind="ExternalInput").ap()
        wqkb_d = nc.dram_tensor("wqkb", [C, 4 * 128], BF16,
                                kind="ExternalInput").ap()
    wqk_d = nc.dram_tensor("wqk", [C, 4 * 128], XDT, kind="ExternalInput").ap()
    wv_d = nc.dram_tensor("wv", [C, HPC * HD], BF16, kind="ExternalInput").ap()
    wo_d = nc.dram_tensor("wo", [HPC * HD, C], BF16, kind="ExternalInput").ap()
    mask_d = nc.dram_tensor("masks", [n_tiles, KB, 2, QB], PDT,
                            kind="ExternalInput").ap()
    bqk_d = nc.dram_tensor("bqk", [128, 4], F32, kind="ExternalInput").ap()
    bv_d = nc.dram_tensor("bv", [128, 2], F32, kind="ExternalInput").ap()
    out_d = nc.dram_tensor("out", [T, C], BF16, kind="ExternalOutput").ap()

    with tile.TileContext(nc) as tc:
        with tc.tile_pool(name="w", bufs=1) as wpool, \
             tc.tile_pool(name="psS", bufs=2, space="PSUM") as spool, \
             tc.tile_pool(name="psO", bufs=1, space="PSUM") as bpool, \
             tc.tile_pool(name="psJ", bufs=2, space="PSUM") as jpool, \
             tc.tile_pool(name="ptp", bufs=4) as ptp, \
             tc.tile_pool(name="divp", bufs=2) as divp, \
             tc.tile_pool(name="otp", bufs=3) as otp:
            # ---- resident SBUF tensors ----
            xt = wpool.tile([128, CI, T], XDT)         # x^T for q/k proj
            if FP8_PROJ:
                xtb = wpool.tile([128, CI, T], BF16)   # x^T, v + ts0 q/k
                wqkb = wpool.tile([128, CI, 512], BF16)
            else:
                xtb = xt
            wqk = wpool.tile([128, CI, 512], XDT)
            if not FP8_PROJ:
                wqkb = wqk
            wv = wpool.tile([128, CI, HPC * HD], BF16)
            wo = wpool.tile([128, 2, C], BF16)         # head-pair chunks
            masks = wpool.tile([128, n_tiles, 2, QB], PDT)
            bqk = wpool.tile([128, 4], F32)
            bv = wpool.tile([128, 2], F32)
            ebias = wpool.tile([128, 1], F32)
            nc.gpsimd.memset(ebias[:], EBIAS)
            # q tiles hold (q_hA | q_hB) on partitions 0-63 / 64-127.
            # k is stored zero-padded per head (other head's partitions are
            # zero) so S matmuls present K=128 to the PE.
            qp = [wpool.tile([128, T], BF16, tag=f"qp{i}", name=f"qp{i}")
                  for i in range(2)]
            kz = [wpool.tile([128, T], BF16, tag=f"kz{i}", name=f"kz{i}")
                  for i in range(4)]          # index = 2*pair + head
            vaug = wpool.tile([128, NJ, HPC * (HD + 1)], PDT)
            attn = [wpool.tile([128, T], BF16, tag=f"attn{p}",
                               name=f"attn{p}") for p in range(2)]

            # ---- DMA preload: few big strided transfers, ordered so the
            # first projection group can start after ~1.5MB ----
            xtv = xt_d.rearrange("(ci p) t -> p ci t", p=128)
            wqkv = wqk_d.rearrange("(ci p) n -> p ci n", p=128)
            wvv = wv_d.rearrange("(ci p) n -> p ci n", p=128)
            wov = wo_d.rearrange("(k p) n -> p k n", p=128)
            maskv = mask_d.rearrange("n p h q -> p n h q")
            if FP8_PROJ:
                xtbv = xtb_d.rearrange("(ci p) t -> p ci t", p=128)
                wqkbv = wqkb_d.rearrange("(ci p) n -> p ci n", p=128)
                # ts0 projects in bf16: its inputs lead the DMA order
                nc.sync.dma_start(wqkb[:, :, 0:256], wqkbv[:, :, 0:256])
                nc.sync.dma_start(xtb[:, :, 0:512], xtbv[:, :, 0:512])
                nc.sync.dma_start(wqkb[:, :, 256:512], wqkbv[:, :, 256:512])
            else:
                nc.sync.dma_start(wqk[:, :, 0:256], wqkv[:, :, 0:256])
                nc.sync.dma_start(xt[:, :, 0:512], xtv[:, :, 0:512])
                nc.sync.dma_start(wqk[:, :, 256:512], wqkv[:, :, 256:512])
            # k zero-padding via gpsimd (Pool idle during the lead-in)
            for p in range(2):
                nc.gpsimd.memset(kz[2 * p][64:128, :], 0.0)
                nc.gpsimd.memset(kz[2 * p + 1][0:64, :], 0.0)
            va = vaug[:].rearrange("p j (h d) -> p j h d", h=HPC)
            nc.vector.tensor_copy(
                va[:, :, :, HD:HD + 1],
                nc.const_aps.tensor(1.0, (128, NJ, HPC, 1)))
            nc.sync.dma_start(wv[:], wvv[:])
            if FP8_PROJ:
                nc.sync.dma_start(wqk[:], wqkv[:])
            for qn in range(1, 4):
                nc.sync.dma_start(xt[:, :, qn * 512:(qn + 1) * 512],
                                  xtv[:, :, qn * 512:(qn + 1) * 512])
                if FP8_PROJ:
                    nc.sync.dma_start(xtb[:, :, qn * 512:(qn + 1) * 512],
                                      xtbv[:, :, qn * 512:(qn + 1) * 512])
            nc.sync.dma_start(wo[:], wov[:])
            nc.sync.dma_start(masks[:], maskv[:])
            nc.sync.dma_start(bqk[:], bqk_d)
            nc.sync.dma_start(bv[:], bv_d)

            # ---- emit helpers (dedup'd so "ensure" calls are idempotent) --
            done = set()
            fillers = []

            def emit_qk(co, ts):
                # q/k projection group: 128 output channels x 512 t columns
                key = ("qk", co, ts)
                if key in done:
                    return False
                done.add(key)
                sl = slice(ts * 512, (ts + 1) * 512)
                ps = jpool.tile([128, 512], F32, tag="pj", name="pj")
                if FP8_PROJ and ts > 0:
                    for cp in range(CI // 2):
                        nc.tensor.matmul(
                            ps[:],
                            wqk[:, 2 * cp:2 * cp + 2,
                                co * 128:(co + 1) * 128],
                            xt[:, 2 * cp:2 * cp + 2, sl],
                            start=(cp == 0), stop=(cp == CI // 2 - 1),
                            perf_mode=DR)
                else:
                    # ts0 in bf16: causal rows 0-511 see few keys, so their
                    # softmax cannot average away fp8 logit noise
                    for ci in range(CI):
                        nc.tensor.matmul(
                            ps[:], wqkb[:, ci, co * 128:(co + 1) * 128],
                            xtb[:, ci, sl], start=(ci == 0),
                            stop=(ci == CI - 1))
                pair, is_k = co // 2, co % 2
                if is_k:
                    dsts = [(kz[2 * pair][0:64, sl], ps[0:64, :],
                             bqk[0:64, co:co + 1]),
                            (kz[2 * pair + 1][64:128, sl], ps[64:128, :],
                             bqk[64:128, co:co + 1])]
                else:
                    dsts = [(qp[pair][:, sl], ps[:], bqk[:, co:co + 1])]
                for dst_ap, src_ap, b_ap in dsts:
                    if apply_qk_bias:
                        nc.scalar.activation(dst_ap, src_ap, AF.Identity,
                                             bias=b_ap, scale=1.0)
                    else:
                        nc.vector.tensor_copy(dst_ap, src_ap)
                return True

            def emit_v(tj):
                key = ("v", tj)
                if key in done:
                    return False
                done.add(key)
                ps = jpool.tile([128, 512], F32, tag="pj", name="pj")
                for ci in range(CI):
                    nc.tensor.matmul(
                        ps[:, 0:HPC * HD],
                        xtb[:, ci, tj * 128:(tj + 1) * 128],
                        wv[:, ci, :], start=(ci == 0),
                        stop=(ci == CI - 1))
                nc.vector.tensor_copy(
                    va[:, tj, :, 0:HD],
                    ps[:, 0:HPC * HD].rearrange("p (h d) -> p h d", h=HPC))
                return True

            def emit_outproj(ts, nk, tail=False):
                # partial out rows [ts*128, (ts+1)*128), cols [nk*512, ...)
                ps = jpool.tile([128, 512], F32, tag="pj", name="pj")
                for pair in range(2):
                    nc.tensor.matmul(
                        ps[:], attn[pair][:, ts * 128:(ts + 1) * 128],
                        wo[:, pair, nk * 512:(nk + 1) * 512],
                        start=(pair == 0), stop=(pair == 1))
                ot = otp.tile([128, 512], BF16, tag="ot", name="ot")
                if tail and nk == 1:
                    # the Act engine is free of exp work at the tail
                    nc.scalar.activation(ot[:], ps[:], AF.Copy, 0.0,
                                         scale=1.0)
                else:
                    nc.vector.tensor_copy(ot[:], ps[:])
                nc.sync.dma_start(
                    out_d[ts * 128:(ts + 1) * 128, nk * 512:(nk + 1) * 512],
                    ot[:])
                return True

            def poll():
                # pop until one filler emits real work (dedup'd ones no-op)
                while fillers:
                    if fillers.pop(0)():
                        return

            # ---- attention (S^T layout), software-pipelined ----
            for pair in range(2):
                q_t = qp[pair]
                for s in range(NSLAB):
                    chunks = _chunk_list(btype, s)
                    i_lo = s * BPS
                    # force any projections this slab needs (usually already
                    # pulled in as fillers during the previous slab)
                    emit_qk(2 * pair, s)
                    for (j, _, _) in chunks:
                        emit_qk(2 * pair + 1, j // 4)
                        emit_v(j)
                    # queue filler work for the chunk loop
                    if s + 1 < NSLAB:
                        for (j, _, _) in _chunk_list(btype, s + 1):
                            fillers.append(
                                lambda p=pair, jj=j: emit_qk(2 * p + 1,
                                                             jj // 4))
                            fillers.append(lambda jj=j: emit_v(jj))
                        fillers.append(
                            lambda p=pair, ss=s + 1: emit_qk(2 * p, ss))
                    elif pair == 0:
                        for s2 in range(NSLAB):
                            fillers.append(lambda s2=s2: emit_qk(2, s2))
                            fillers.append(lambda s2=s2: emit_qk(3, s2))

                    out_ps = [bpool.tile([HD + 1, SLAB], F32,
                                         tag=f"outps{_hl}",
                                         name=f"outps{_hl}", bufs=1)
                              for _hl in range(2)]
                    written = np.zeros(BPS, dtype=bool)

                    def emit_pv(unit, base_i0, i1u, pt, last):
                        # unit: 1 chunk (plain matmul) or a (2jp, 2jp+1)
                        # chunk pair (fp8 DoubleRow over both k-chunks)
                        r0 = base_i0 - i_lo
                        segs = []
                        c = r0 * QB
                        end = (i1u - i_lo + 1) * QB
                        while c < end:
                            st = written[c // QB]
                            cc = c + QB
                            while cc < end and written[cc // QB] == st:
                                cc += QB
                            segs.append((c, cc, not st))
                            c = cc
                        for hl in range(2):
                            hh = 2 * pair + hl
                            for (c0, c1, st_flag) in segs:
                                r_lo = c0 - r0 * QB
                                r_hi = c1 - r0 * QB
                                if FP8_PV and len(unit) == 2:
                                    jp = unit[0][0] // 2
                                    nc.tensor.matmul(
                                        out_ps[hl][:, c0:c1],
                                        vaug[:, 2 * jp:2 * jp + 2,
                                             hh * (HD + 1):
                                             (hh + 1) * (HD + 1)],
                                        pt[:, hl, :, r_lo:r_hi],
                                        start=st_flag, stop=last,
                                        perf_mode=DR, skip_group_check=True)
                                elif FP8_PV:
                                    nc.tensor.matmul(
                                        out_ps[hl][:, c0:c1],
                                        vaug[:, unit[0][0], hh * (HD + 1):
                                             (hh + 1) * (HD + 1)],
                                        pt[:, hl, 0, r_lo:r_hi],
                                        start=st_flag, stop=last,
                                        skip_group_check=True)
                                else:
                                    nc.tensor.matmul(
                                        out_ps[hl][:, c0:c1],
                                        vaug[:, unit[0][0], hh * (HD + 1):
                                             (hh + 1) * (HD + 1)],
                                        pt[:, hl, r_lo:r_hi],
                                        start=st_flag, stop=last,
                                        skip_group_check=True)
                        for rr in range(r0, i1u - i_lo + 1):
                            written[rr] = True

                    # group chunks into DoubleRow-able (even j, odd j) pairs
                    units = []
                    k = 0
                    while k < len(chunks):
                        a = chunks[k]
                        if (FP8_PV and k + 1 < len(chunks)
                                and a[0] % 2 == 0
                                and chunks[k + 1][0] == a[0] + 1):
                            units.append([a, chunks[k + 1]])
                            k += 2
                        else:
                            units.append([a])
                            k += 1

                    pend = None
                    for unit in units:
                        base_i0 = unit[0][1]
                        i1u = unit[0][2]
                        if FP8_PV:
                            pt = ptp.tile([128, 2, 2, SLAB], PDT, tag="pt",
                                          name="pt")
                        else:
                            pt = ptp.tile([128, 2, SLAB], PDT, tag="pt",
                                          name="pt")
                        for uidx, (j, i0, i1) in enumerate(unit):
                            n_cols = (i1 - i0 + 1) * QB
                            off = (i0 - base_i0) * QB
                            sps = spool.tile([128, 2, SLAB], F32, tag="sst",
                                             name="sst", bufs=2)
                            for hl in range(2):
                                nc.tensor.matmul(
                                    sps[:, hl, 0:n_cols],
                                    kz[2 * pair + hl][:,
                                                      j * KB:(j + 1) * KB],
                                    q_t[:, i0 * QB:i0 * QB + n_cols],
                                    start=True, stop=True)
                            if FP8_PV:
                                if uidx == 1 and off > 0:
                                    nc.gpsimd.memset(pt[:, :, 1, 0:off], 0.0)
                                dst = pt[:, :, uidx, off:off + n_cols]
                            else:
                                dst = pt[:, :, 0:n_cols]
                            nc.scalar.activation(dst, sps[:, :, 0:n_cols],
                                                 AF.Exp, bias=ebias[:],
                                                 scale=ESCALE)
                            for i in range(i0, i1 + 1):
                                col = (i - base_i0) * QB
                                if FP8_PV:
                                    reg = pt[:, :, uidx, col:col + QB]
                                else:
                                    reg = pt[:, :, col:col + QB]
                                if btype[j, i] == 2:
                                    ti = tidx[j, i]
                                    nc.vector.tensor_tensor(
                                        out=reg, in0=reg,
                                        in1=masks[:, ti, :, :], op=MULT)
                                elif btype[j, i] == 1:
                                    nc.gpsimd.memset(reg, 0.0)
                            poll()
                        if pend is not None:
                            emit_pv(*pend, last=False)
                        pend = (unit, base_i0, i1u, pt)
                    emit_pv(*pend, last=True)

                    # pre-ensure the next slab's q/k (and first v) before the
                    # division: their PSUM->SBUF copies must land ahead of
                    # the division chain in the DVE queue, or the next S
                    # matmul stalls behind ~3us of division work
                    if s + 1 < NSLAB:
                        nxt = _chunk_list(btype, s + 1)
                        emit_qk(2 * pair, s + 1)
                        for t2 in sorted({j2 // 4 for (j2, _, _) in nxt}):
                            emit_qk(2 * pair + 1, t2)
                        emit_v(nxt[0][0])
                    elif pair == 0:
                        nxt = _chunk_list(btype, 0)
                        emit_qk(2, 0)
                        for t2 in sorted({j2 // 4 for (j2, _, _) in nxt}):
                            emit_qk(3, t2)

                    # per-slab softmax division: 1/den from the augmented
                    # ones-row, broadcast to 128 partitions, applied in attn.
                    # NOTE: reciprocal_approx_fast needs a partition-0 SBUF
                    # input and tensor_tensor needs matching partition bases
                    # on HW — sim accepts more than hardware does here.
                    sums = divp.tile([1, 2 * SLAB], F32, tag="sums",
                                     name="sums")
                    for hl in range(2):
                        nc.vector.tensor_copy(
                            sums[:, hl * SLAB:(hl + 1) * SLAB],
                            out_ps[hl][HD:HD + 1, :])
                    rec1 = divp.tile([1, 2 * SLAB], F32, tag="rec1",
                                     name="rec1")
                    nc.vector.reciprocal_approx_fast(rec1[:], sums[:])
                    # attn copies overlap the Pool broadcast below
                    for hl in range(2):
                        nc.vector.tensor_copy(
                            attn[pair][64 * hl:64 * hl + 64,
                                       s * SLAB:(s + 1) * SLAB],
                            out_ps[hl][0:HD, :])
                    rec128 = divp.tile([128, 2 * SLAB], F32, tag="rec128",
                                       name="rec128")
                    nc.gpsimd.partition_broadcast(rec128[:], rec1[:])

                    def divide(hl, c0, c1):
                        # attn[rows hl] cols [s*SLAB+c0, s*SLAB+c1), in place
                        dst = attn[pair][64 * hl:64 * hl + 64,
                                         s * SLAB + c0:s * SLAB + c1]
                        nc.vector.tensor_tensor(
                            out=dst, in0=dst,
                            in1=rec128[64 * hl:64 * hl + 64,
                                       hl * SLAB + c0:hl * SLAB + c1],
                            op=MULT)
                        if apply_v_bias:
                            nc.vector.tensor_scalar(
                                out=dst, in0=dst,
                                scalar1=bv[64 * hl:64 * hl + 64,
                                           pair:pair + 1],
                                scalar2=None, op0=mybir.AluOpType.add)

                    for hl in range(2):
                        divide(hl, 0, SLAB)
                    if pair == 1:
                        if s + 1 < NSLAB:
                            # this slab's output projection overlaps the
                            # next slab's attention
                            for ts in range(s * BPS, (s + 1) * BPS):
                                for nk in range(2):
                                    fillers.append(
                                        lambda ts=ts, nk=nk:
                                        emit_outproj(ts, nk))
                        else:
                            for ts in range(s * BPS, (s + 1) * BPS):
                                emit_outproj(ts, 0, tail=True)
                                emit_outproj(ts, 1, tail=True)

            # drain any unpulled fillers (small slabs / non-causal masks)
            while fillers:
                fillers.pop(0)()

    nc.compile()
    return nc


def _get_program(mask_bool, apply_qk_bias, apply_v_bias):
    key = (mask_bool.tobytes(), apply_qk_bias, apply_v_bias)
    if key not in _cache:
        btype, tidx, tiles = _build_plan(mask_bool)
        nc = _build_program(btype, tidx, len(tiles), apply_qk_bias,
                            apply_v_bias)
        _cache[key] = (nc, tiles)
    return _cache[key]


def _core_in_map(c, xts, Wqkv, bqkv, Wo, masks_arr):
    import ml_dtypes

    BF = ml_dtypes.bfloat16
    XDT = ml_dtypes.float8_e4m3 if FP8_PROJ else BF
    # Wqk prescaled so fp8 sees ~N(0,1) weights (denormals hurt); the x32
    # per q and k is divided back out in the exp scale (SCALE/1024)
    WSCALE = 32.0 if FP8_PROJ else 1.0
    b, g = divmod(c, NCORES // B)
    hs = [HPC * g + i for i in range(HPC)]
    # wqk column chunks: [q_h0|q_h1, k_h0|k_h1, q_h2|q_h3, k_h2|k_h3]
    cols, bias_cols = [], []
    for pair in range(2):
        ha, hb = hs[2 * pair], hs[2 * pair + 1]
        for base in (0, C):  # q then k offset in Wqkv columns
            cols.append(Wqkv[:, base + ha * HD:base + (ha + 1) * HD])
            cols.append(Wqkv[:, base + hb * HD:base + (hb + 1) * HD])
            bias_cols.append(np.concatenate([
                bqkv[base + ha * HD:base + (ha + 1) * HD],
                bqkv[base + hb * HD:base + (hb + 1) * HD]]) * WSCALE)
    wqk_c = (np.concatenate(cols, axis=1) * WSCALE).astype(XDT)
    bqk_c = np.stack(bias_cols, axis=1).astype(np.float32)
    wv_c = np.concatenate(
        [Wqkv[:, 2 * C + h * HD:2 * C + (h + 1) * HD] for h in hs],
        axis=1).astype(BF)
    wo_c = np.concatenate(
        [Wo[h * HD:(h + 1) * HD, :] for h in hs], axis=0).astype(BF)
    bv_c = np.zeros((128, 2), dtype=np.float32)
    for pair in range(2):
        ha, hb = hs[2 * pair], hs[2 * pair + 1]
        bv_c[0:HD, pair] = bqkv[2 * C + ha * HD:2 * C + (ha + 1) * HD]
        bv_c[HD:128, pair] = bqkv[2 * C + hb * HD:2 * C + (hb + 1) * HD]
    m = {
        "xt": xts[b][0], "wqk": wqk_c, "wv": wv_c, "wo": wo_c,
        "masks": masks_arr, "bqk": bqk_c, "bv": bv_c,
    }
    if FP8_PROJ:
        m["xtb"] = xts[b][1]
        m["wqkb"] = (np.concatenate(cols, axis=1) * WSCALE).astype(BF)
    return m


def _prep_shared(x, tiles):
    import ml_dtypes

    BF = ml_dtypes.bfloat16
    XDT = ml_dtypes.float8_e4m3 if FP8_PROJ else BF
    PDT = ml_dtypes.float8_e4m3 if FP8_PV else BF
    # per batch: (x^T for q/k proj, x^T for v proj — bf16 copy under fp8)
    xts = []
    for b in range(B):
        xtT = np.ascontiguousarray(x[b].T)
        xts.append((xtT.astype(XDT),
                    xtT.astype(BF) if FP8_PROJ else None))
    # masks pre-doubled for the two heads sharing one exp: [n, 128, 2, 128]
    masks_arr = np.ascontiguousarray(
        np.stack([np.stack([t, t], axis=1) for t in tiles])).astype(PDT)
    return xts, masks_arr


def kernel(x, attention_mask, Wqkv, bqkv, Wo, bo, _trace=False):
    from concourse.bass_utils import run_bass_kernel_spmd

    x = np.asarray(x, dtype=np.float32)
    mask_bool = np.asarray(attention_mask)[0, 0] != 0
    Wqkv = np.asarray(Wqkv, dtype=np.float32)
    bqkv = np.asarray(bqkv, dtype=np.float32)
    Wo = np.asarray(Wo, dtype=np.float32)
    bo = np.asarray(bo, dtype=np.float32)

    apply_qk_bias = bool(np.any(bqkv[:2 * C]))
    apply_v_bias = bool(np.any(bqkv[2 * C:]))
    nc, tiles = _get_program(mask_bool, apply_qk_bias, apply_v_bias)

    xts, masks_arr = _prep_shared(x, tiles)
    in_maps = [_core_in_map(c, xts, Wqkv, bqkv, Wo, masks_arr)
               for c in range(NCORES)]

    kwargs = {}
    if _trace:
        kwargs = dict(trace=True, trace_cores=[0])
    res = run_bass_kernel_spmd(nc, in_maps, core_ids=list(range(NCORES)),
                               **kwargs)
    out = np.empty((B, T, C), dtype=np.float32)
    gpb = NCORES // B
    for b in range(B):
        acc = res.results[b * gpb]["out"].astype(np.float32)
        for g in range(1, gpb):
            acc = acc + res.results[b * gpb + g]["out"].astype(np.float32)
        out[b] = acc + bo
    if _trace:
        kernel._last_results = res
    return out


# revision 58
# speedup vs baseline: 1.0143x; 1.0143x over previous
"""Multi-head self-attention (B=2, T=2048, C=1024, H=16) on 8 trn2 cores.

Sharding: core c -> batch b = c//4, heads 4*(c%4) .. 4*(c%4)+3.
Each core: QKV projection for its 4 heads, causal attention in S^T layout
(keys on partitions), partial output projection over its heads' rows of Wo.
Host sums the 4 partials per batch element and adds bo.

Matmul inputs are bf16 (PSUM accumulates fp32), except the q/k projections
for t >= 512, which run in fp8e4m3 with DoubleRow (2x PE rate; Wqk
prescaled x32 host-side, folded back via the exp scale).  fp8 there only
perturbs logits, which softmax averages away for rows with many keys; the
first 512 rows (few-key softmax) and the whole v path stay bf16.

The schedule keeps the PE continuously busy: inputs arrive as a few large
strided DMAs ordered so the first projection group starts ~4us in,
projections for slab s+1 / the other head-pair / output-projection pieces
are interleaved as "filler" between software-pipelined attention chunks
(S-matmul of chunk c+1 issues before PV of chunk c, so the PE never waits
on the Act engine's exp), the softmax division runs per slab with its
PSUM readers front-loaded, and the output projection for slab s overlaps
pair-1's slab s+1 attention.

HW-validated pitfalls baked in: reciprocal_approx_fast needs a partition-0
SBUF input; two-SBUF-input DVE ops need matching partition bases; gpsimd
cannot touch PSUM (CoreSim accepts all of these, hardware does not).
"""
import sys

sys.path.insert(0, "/opt/trn_rl_repo")

import numpy as np

B, T, C, H = 2, 2048, 1024, 16
HD = C // H            # 64
NCORES = 8
HPC = H // (NCORES // B)   # heads per core = 4
QB = 128               # q block (columns of S^T)
KB = 128               # k chunk (partitions of S^T)
NJ = T // KB           # 16
NI = T // QB           # 16
SLAB = 512             # q columns processed per attention pass
NSLAB = T // SLAB      # 4
BPS = SLAB // QB       # q blocks per slab = 4
CI = C // 128          # 8 contraction chunks for projections
SCALE = HD ** -0.5
# fp8e4m3 + DoubleRow (2x PE rate) for the q/k projections only (Wqkv
# prescaled x32, folded back via the exp scale).  Quantization there only
# perturbs logits, which softmax dampens.  The v path (projection, PV) stays
# bf16 — v errors hit the output directly and early causal rows have no
# softmax averaging to wash them out — so x is staged twice: fp8 for q/k,
# bf16 for v.
FP8_PROJ = True
FP8_PV = False

_cache = {}


def _build_plan(mask_bool: np.ndarray):
    """mask_bool: [T, T] (q, k). Returns per (j, i) block types and tiles.

    type 0 = all valid (no mask work), 1 = all masked (skip), 2 = mixed.
    Tiles are stored transposed to match S^T ([k_local, q_local])."""
    btype = np.zeros((NJ, NI), dtype=np.int32)
    tidx = np.full((NJ, NI), -1, dtype=np.int32)
    tiles = []
    tile_map = {}
    for j in range(NJ):
        for i in range(NI):
            sub = mask_bool[i * QB:(i + 1) * QB, j * KB:(j + 1) * KB]
            if sub.all():
                btype[j, i] = 0
            elif not sub.any():
                btype[j, i] = 1
            else:
                btype[j, i] = 2
                key = sub.tobytes()
                if key not in tile_map:
                    tile_map[key] = len(tiles)
                    tiles.append(sub.T.astype(np.float32))
                tidx[j, i] = tile_map[key]
    if not tiles:
        tiles.append(np.ones((KB, QB), dtype=np.float32))
    return btype, tidx, np.stack(tiles)


def _chunk_list(btype, s):
    """Live (j, i0, i1) chunk runs for slab s."""
    i_lo, i_hi = s * BPS, (s + 1) * BPS
    chunks = []
    for j in range(NJ):
        live = [i for i in range(i_lo, i_hi) if btype[j, i] != 1]
        if live:
            chunks.append((j, min(live), max(live)))
    return chunks


def _build_program(btype, tidx, n_tiles, apply_qk_bias, apply_v_bias):
    import concourse.tile as tile
    import concourse.mybir as mybir
    from concourse import bacc

    F32 = mybir.dt.float32
    BF16 = mybir.dt.bfloat16
    F8 = mybir.dt.float8e4
    XDT = F8 if FP8_PROJ else BF16
    PDT = F8 if FP8_PV else BF16
    DR = mybir.MatmulPerfMode.DoubleRow
    ESCALE = SCALE / 1024.0 if FP8_PROJ else SCALE
    # shift logits so exp() stays well inside fp8e4m3 range; the softmax
    # ratio is shift-invariant (numerator and denominator share e^-3)
    EBIAS = -3.0 if FP8_PV else 0.0
    AF = mybir.ActivationFunctionType
    MULT = mybir.AluOpType.mult

    nc = bacc.Bacc("TRN2", target_bir_lowering=False, debug=False)
    xt_d = nc.dram_tensor("xt", [C, T], XDT, kind="ExternalInput").ap()
    if FP8_PROJ:
        xtb_d = nc.dram_tensor("xtb", [C, T], BF16,
                               kind="ExternalInput").ap()
        wqkb_d = nc.dram_tensor("wqkb", [C, 4 * 128], BF16,
                                kind="ExternalInput").ap()
    wqk_d = nc.dram_tensor("wqk", [C, 4 * 128], XDT, kind="ExternalInput").ap()
    wv_d = nc.dram_tensor("wv", [C, HPC * HD], BF16, kind="ExternalInput").ap()
    wo_d = nc.dram_tensor("wo", [HPC * HD, C], BF16, kind="ExternalInput").ap()
    mask_d = nc.dram_tensor("masks", [n_tiles, KB, 2, QB], PDT,
                            kind="ExternalInput").ap()
    bqk_d = nc.dram_tensor("bqk", [128, 4], F32, kind="ExternalInput").ap()
    bv_d = nc.dram_tensor("bv", [128, 2], F32, kind="ExternalInput").ap()
    out_d = nc.dram_tensor("out", [T, C], BF16, kind="ExternalOutput").ap()

    with tile.TileContext(nc) as tc:
        with tc.tile_pool(name="w", bufs=1) as wpool, \
             tc.tile_pool(name="psS", bufs=2, space="PSUM") as spool, \
             tc.tile_pool(name="psO", bufs=1, space="PSUM") as bpool, \
             tc.tile_pool(name="psJ", bufs=2, space="PSUM") as jpool, \
             tc.tile_pool(name="ptp", bufs=4) as ptp, \
             tc.tile_pool(name="divp", bufs=2) as divp, \
             tc.tile_pool(name="otp", bufs=3) as otp:
            # ---- resident SBUF tensors ----
            xt = wpool.tile([128, CI, T], XDT)         # x^T for q/k proj
            if FP8_PROJ:
                xtb = wpool.tile([128, CI, T], BF16)   # x^T, v + ts0 q/k
                wqkb = wpool.tile([128, CI, 512], BF16)
            else:
                xtb = xt
            wqk = wpool.tile([128, CI, 512], XDT)
            if not FP8_PROJ:
                wqkb = wqk
            wv = wpool.tile([128, CI, HPC * HD], BF16)
            wo = wpool.tile([128, 2, C], BF16)         # head-pair chunks
            masks = wpool.tile([128, n_tiles, 2, QB], PDT)
            bqk = wpool.tile([128, 4], F32)
            bv = wpool.tile([128, 2], F32)
            ebias = wpool.tile([128, 1], F32)
            nc.gpsimd.memset(ebias[:], EBIAS)
            # q tiles hold (q_hA | q_hB) on partitions 0-63 / 64-127.
            # k is stored zero-padded per head (other head's partitions are
            # zero) so S matmuls present K=128 to the PE.
            qp = [wpool.tile([128, T], BF16, tag=f"qp{i}", name=f"qp{i}")
                  for i in range(2)]
            kz = [wpool.tile([128, T], BF16, tag=f"kz{i}", name=f"kz{i}")
                  for i in range(4)]          # index = 2*pair + head
            vaug = wpool.tile([128, NJ, HPC * (HD + 1)], PDT)
            attn = [wpool.tile([128, T], BF16, tag=f"attn{p}",
                               name=f"attn{p}") for p in range(2)]

            # ---- DMA preload: few big strided transfers, ordered so the
            # first projection group can start after ~1.5MB ----
            xtv = xt_d.rearrange("(ci p) t -> p ci t", p=128)
            wqkv = wqk_d.rearrange("(ci p) n -> p ci n", p=128)
            wvv = wv_d.rearrange("(ci p) n -> p ci n", p=128)
            wov = wo_d.rearrange("(k p) n -> p k n", p=128)
            maskv = mask_d.rearrange("n p h q -> p n h q")
            if FP8_PROJ:
                xtbv = xtb_d.rearrange("(ci p) t -> p ci t", p=128)
                wqkbv = wqkb_d.rearrange("(ci p) n -> p ci n", p=128)
                # ts0 projects in bf16: its inputs lead the DMA order
                nc.sync.dma_start(wqkb[:, :, 0:256], wqkbv[:, :, 0:256])
                nc.sync.dma_start(xtb[:, :, 0:512], xtbv[:, :, 0:512])
                nc.sync.dma_start(wqkb[:, :, 256:512], wqkbv[:, :, 256:512])
            else:
                nc.sync.dma_start(wqk[:, :, 0:256], wqkv[:, :, 0:256])
                nc.sync.dma_start(xt[:, :, 0:512], xtv[:, :, 0:512])
                nc.sync.dma_start(wqk[:, :, 256:512], wqkv[:, :, 256:512])
            # k zero-padding via gpsimd (Pool idle during the lead-in)
            for p in range(2):
                nc.gpsimd.memset(kz[2 * p][64:128, :], 0.0)
                nc.gpsimd.memset(kz[2 * p + 1][0:64, :], 0.0)
            va = vaug[:].rearrange("p j (h d) -> p j h d", h=HPC)
            nc.vector.tensor_copy(
                va[:, :, :, HD:HD + 1],
                nc.const_aps.tensor(1.0, (128, NJ, HPC, 1)))
            nc.sync.dma_start(wv[:], wvv[:])
            if FP8_PROJ:
                nc.sync.dma_start(wqk[:], wqkv[:])
            for qn in range(1, 4):
                nc.sync.dma_start(xt[:, :, qn * 512:(qn + 1) * 512],
                                  xtv[:, :, qn * 512:(qn + 1) * 512])
                if FP8_PROJ:
                    nc.sync.dma_start(xtb[:, :, qn * 512:(qn + 1) * 512],
                                      xtbv[:, :, qn * 512:(qn + 1) * 512])
            nc.sync.dma_start(wo[:], wov[:])
            nc.sync.dma_start(masks[:], maskv[:])
            nc.sync.dma_start(bqk[:], bqk_d)
            nc.sync.dma_start(bv[:], bv_d)

            # ---- emit helpers (dedup'd so "ensure" calls are idempotent) --
            done = set()
            fillers = []

            def emit_qk(co, ts):
                # q/k projection group: 128 output channels x 512 t columns
                key = ("qk", co, ts)
                if key in done:
                    return False
                done.add(key)
                sl = slice(ts * 512, (ts + 1) * 512)
                ps = jpool.tile([128, 512], F32, tag="pj", name="pj")
                if FP8_PROJ and ts > 0:
                    for cp in range(CI // 2):
                        nc.tensor.matmul(
                            ps[:],
                            wqk[:, 2 * cp:2 * cp + 2,
                                co * 128:(co + 1) * 128],
                            xt[:, 2 * cp:2 * cp + 2, sl],
                            start=(cp == 0), stop=(cp == CI // 2 - 1),
                            perf_mode=DR)
                else:
                    # ts0 in bf16: causal rows 0-511 see few keys, so their
                    # softmax cannot average away fp8 logit noise
                    for ci in range(CI):
                        nc.tensor.matmul(
                            ps[:], wqkb[:, ci, co * 128:(co + 1) * 128],
                            xtb[:, ci, sl], start=(ci == 0),
                            stop=(ci == CI - 1))
                pair, is_k = co // 2, co % 2
                if is_k:
                    dsts = [(kz[2 * pair][0:64, sl], ps[0:64, :],
                             bqk[0:64, co:co + 1]),
                            (kz[2 * pair + 1][64:128, sl], ps[64:128, :],
                             bqk[64:128, co:co + 1])]
                else:
                    dsts = [(qp[pair][:, sl], ps[:], bqk[:, co:co + 1])]
                for dst_ap, src_ap, b_ap in dsts:
                    if apply_qk_bias:
                        nc.scalar.activation(dst_ap, src_ap, AF.Identity,
                                             bias=b_ap, scale=1.0)
                    else:
                        nc.vector.tensor_copy(dst_ap, src_ap)
                return True

            def emit_v(tj):
                key = ("v", tj)
                if key in done:
                    return False
                done.add(key)
                ps = jpool.tile([128, 512], F32, tag="pj", name="pj")
                for ci in range(CI):
                    nc.tensor.matmul(
                        ps[:, 0:HPC * HD],
                        xtb[:, ci, tj * 128:(tj + 1) * 128],
                        wv[:, ci, :], start=(ci == 0),
                        stop=(ci == CI - 1))
                nc.vector.tensor_copy(
                    va[:, tj, :, 0:HD],
                    ps[:, 0:HPC * HD].rearrange("p (h d) -> p h d", h=HPC))
                return True

            def emit_outproj(ts, nk, tail=False):
                # partial out rows [ts*128, (ts+1)*128), cols [nk*512, ...)
                ps = jpool.tile([128, 512], F32, tag="pj", name="pj")
                for pair in range(2):
                    nc.tensor.matmul(
                        ps[:], attn[pair][:, ts * 128:(ts + 1) * 128],
                        wo[:, pair, nk * 512:(nk + 1) * 512],
                        start=(pair == 0), stop=(pair == 1))
                ot = otp.tile([128, 512], BF16, tag="ot", name="ot")
                if tail and nk == 1:
                    # the Act engine is free of exp work at the tail
                    nc.scalar.activation(ot[:], ps[:], AF.Copy, 0.0,
                                         scale=1.0)
                else:
                    nc.vector.tensor_copy(ot[:], ps[:])
                nc.sync.dma_start(
                    out_d[ts * 128:(ts + 1) * 128, nk * 512:(nk + 1) * 512],
                    ot[:])
                return True

            def poll():
                # pop until one filler emits real work (dedup'd ones no-op)
                while fillers:
                    if fillers.pop(0)():
                        return

            # ---- attention (S^T layout), software-pipelined ----
            for pair in range(2):
                q_t = qp[pair]
                for s in range(NSLAB):
                    chunks = _chunk_list(btype, s)
                    i_lo = s * BPS
                    # force any projections this slab needs (usually already
                    # pulled in as fillers during the previous slab)
                    emit_qk(2 * pair, s)
                    for (j, _, _) in chunks:
                        emit_qk(2 * pair + 1, j // 4)
                        emit_v(j)
                    # queue filler work for the chunk loop
                    if s + 1 < NSLAB:
                        for (j, _, _) in _chunk_list(btype, s + 1):
                            fillers.append(
                                lambda p=pair, jj=j: emit_qk(2 * p + 1,
                                                             jj // 4))
                            fillers.append(lambda jj=j: emit_v(jj))
                        fillers.append(
                            lambda p=pair, ss=s + 1: emit_qk(2 * p, ss))
                    elif pair == 0:
                        for s2 in range(NSLAB):
                            fillers.append(lambda s2=s2: emit_qk(2, s2))
                            fillers.append(lambda s2=s2: emit_qk(3, s2))

                    out_ps = [bpool.tile([HD + 1, SLAB], F32,
                                         tag=f"outps{_hl}",
                                         name=f"outps{_hl}", bufs=1)
                              for _hl in range(2)]
                    written = np.zeros(BPS, dtype=bool)

                    def emit_pv(unit, base_i0, i1u, pt, last):
                        # unit: 1 chunk (plain matmul) or a (2jp, 2jp+1)
                        # chunk pair (fp8 DoubleRow over both k-chunks)
                        r0 = base_i0 - i_lo
                        segs = []
                        c = r0 * QB
                        end = (i1u - i_lo + 1) * QB
                        while c < end:
                            st = written[c // QB]
                            cc = c + QB
                            while cc < end and written[cc // QB] == st:
                                cc += QB
                            segs.append((c, cc, not st))
                            c = cc
                        for hl in range(2):
                            hh = 2 * pair + hl
                            for (c0, c1, st_flag) in segs:
                                r_lo = c0 - r0 * QB
                                r_hi = c1 - r0 * QB
                                if FP8_PV and len(unit) == 2:
                                    jp = unit[0][0] // 2
                                    nc.tensor.matmul(
                                        out_ps[hl][:, c0:c1],
                                        vaug[:, 2 * jp:2 * jp + 2,
                                             hh * (HD + 1):
                                             (hh + 1) * (HD + 1)],
                                        pt[:, hl, :, r_lo:r_hi],
                                        start=st_flag, stop=last,
                                        perf_mode=DR, skip_group_check=True)
                                elif FP8_PV:
                                    nc.tensor.matmul(
                                        out_ps[hl][:, c0:c1],
                                        vaug[:, unit[0][0], hh * (HD + 1):
                                             (hh + 1) * (HD + 1)],
                                        pt[:, hl, 0, r_lo:r_hi],
                                        start=st_flag, stop=last,
                                        skip_group_check=True)
                                else:
                                    nc.tensor.matmul(
                                        out_ps[hl][:, c0:c1],
                                        vaug[:, unit[0][0], hh * (HD + 1):
                                             (hh + 1) * (HD + 1)],
                                        pt[:, hl, r_lo:r_hi],
                                        start=st_flag, stop=last,
                                        skip_group_check=True)
                        for rr in range(r0, i1u - i_lo + 1):
                            written[rr] = True

                    # group chunks into DoubleRow-able (even j, odd j) pairs
                    units = []
                    k = 0
                    while k < len(chunks):
                        a = chunks[k]
                        if (FP8_PV and k + 1 < len(chunks)
                                and a[0] % 2 == 0
                                and chunks[k + 1][0] == a[0] + 1):
                            units.append([a, chunks[k + 1]])
                            k += 2
                        else:
                            units.append([a])
                            k += 1

                    pend = None
                    for unit in units:
                        base_i0 = unit[0][1]
                        i1u = unit[0][2]
                        if FP8_PV:
                            pt = ptp.tile([128, 2, 2, SLAB], PDT, tag="pt",
                                          name="pt")
                        else:
                            pt = ptp.tile([128, 2, SLAB], PDT, tag="pt",
                                          name="pt")
                        for uidx, (j, i0, i1) in enumerate(unit):
                            n_cols = (i1 - i0 + 1) * QB
                            off = (i0 - base_i0) * QB
                            sps = spool.tile([128, 2, SLAB], F32, tag="sst",
                                             name="sst", bufs=2)
                            for hl in range(2):
                                nc.tensor.matmul(
                                    sps[:, hl, 0:n_cols],
                                    kz[2 * pair + hl][:,
                                                      j * KB:(j + 1) * KB],
                                    q_t[:, i0 * QB:i0 * QB + n_cols],
                                    start=True, stop=True)
                            if FP8_PV:
                                if uidx == 1 and off > 0:
                                    nc.gpsimd.memset(pt[:, :, 1, 0:off], 0.0)
                                dst = pt[:, :, uidx, off:off + n_cols]
                            else:
                                dst = pt[:, :, 0:n_cols]
                            nc.scalar.activation(dst, sps[:, :, 0:n_cols],
                                                 AF.Exp, bias=ebias[:],
                                                 scale=ESCALE)
                            for i in range(i0, i1 + 1):
                                col = (i - base_i0) * QB
                                if FP8_PV:
                                    reg = pt[:, :, uidx, col:col + QB]
                                else:
                                    reg = pt[:, :, col:col + QB]
                                if btype[j, i] == 2:
                                    ti = tidx[j, i]
                                    nc.vector.tensor_tensor(
                                        out=reg, in0=reg,
                                        in1=masks[:, ti, :, :], op=MULT)
                                elif btype[j, i] == 1:
                                    nc.gpsimd.memset(reg, 0.0)
                            poll()
                        if pend is not None:
                            emit_pv(*pend, last=False)
                        pend = (unit, base_i0, i1u, pt)
                    emit_pv(*pend, last=True)

                    # pre-ensure the next slab's q/k (and first v) before the
                    # division: their PSUM->SBUF copies must land ahead of
                    # the division chain in the DVE queue, or the next S
                    # matmul stalls behind ~3us of division work
                    if s + 1 < NSLAB:
                        nxt = _chunk_list(btype, s + 1)
                        emit_qk(2 * pair, s + 1)
                        for t2 in sorted({j2 // 4 for (j2, _, _) in nxt}):
                            emit_qk(2 * pair + 1, t2)
                        emit_v(nxt[0][0])
                    elif pair == 0:
                        nxt = _chunk_list(btype, 0)
                        emit_qk(2, 0)
                        for t2 in sorted({j2 // 4 for (j2, _, _) in nxt}):
                            emit_qk(3, t2)

                    # per-slab softmax division: 1/den from the augmented
                    # ones-row, broadcast to 128 partitions, applied in attn.
                    # NOTE: reciprocal_approx_fast needs a partition-0 SBUF
                    # input and tensor_tensor needs matching partition bases
                    # on HW — sim accepts more than hardware does here.
                    sums = divp.tile([1, 2 * SLAB], F32, tag="sums",
                                     name="sums")
                    for hl in range(2):
                        nc.vector.tensor_copy(
                            sums[:, hl * SLAB:(hl + 1) * SLAB],
                            out_ps[hl][HD:HD + 1, :])
                    # hl0 attn copy rides the idle Act engine (same Copy op
                    # shape as the HW-proven tail casts); hl1 stays on DVE
                    # and overlaps the first broadcast below
                    nc.scalar.activation(
                        attn[pair][0:64, s * SLAB:(s + 1) * SLAB],
                        out_ps[0][0:HD, :], AF.Copy, 0.0, scale=1.0)
                    rec1 = divp.tile([1, 2 * SLAB], F32, tag="rec1",
                                     name="rec1")
                    nc.vector.reciprocal_approx_fast(rec1[:], sums[:])
                    nc.vector.tensor_copy(
                        attn[pair][64:128, s * SLAB:(s + 1) * SLAB],
                        out_ps[1][0:HD, :])
                    rec128 = divp.tile([128, 2 * SLAB], F32, tag="rec128",
                                       name="rec128")
                    # per-head broadcast halves: divide(hl) can start as
                    # soon as its own half lands
                    nc.gpsimd.partition_broadcast(rec128[:, 0:SLAB],
                                                  rec1[:, 0:SLAB])

                    def divide(hl, c0, c1):
                        # attn[rows hl] cols [s*SLAB+c0, s*SLAB+c1), in place
                        dst = attn[pair][64 * hl:64 * hl + 64,
                                         s * SLAB + c0:s * SLAB + c1]
                        nc.vector.tensor_tensor(
                            out=dst, in0=dst,
                            in1=rec128[64 * hl:64 * hl + 64,
                                       hl * SLAB + c0:hl * SLAB + c1],
                            op=MULT)
                        if apply_v_bias:
                            nc.vector.tensor_scalar(
                                out=dst, in0=dst,
                                scalar1=bv[64 * hl:64 * hl + 64,
                                           pair:pair + 1],
                                scalar2=None, op0=mybir.AluOpType.add)

                    divide(0, 0, SLAB)
                    nc.gpsimd.partition_broadcast(rec128[:, SLAB:2 * SLAB],
                                                  rec1[:, SLAB:2 * SLAB])
                    divide(1, 0, SLAB)
                    if pair == 1:
                        if s + 1 < NSLAB:
                            # this slab's output projection overlaps the
                            # next slab's attention
                            for ts in range(s * BPS, (s + 1) * BPS):
                                for nk in range(2):
                                    fillers.append(
                                        lambda ts=ts, nk=nk:
                                        emit_outproj(ts, nk))
                        else:
                            for ts in range(s * BPS, (s + 1) * BPS):
                                emit_outproj(ts, 0, tail=True)
                                emit_outproj(ts, 1, tail=True)

            # drain any unpulled fillers (small slabs / non-causal masks)
            while fillers:
                fillers.pop(0)()

    nc.compile()
    return nc


def _get_program(mask_bool, apply_qk_bias, apply_v_bias):
    key = (mask_bool.tobytes(), apply_qk_bias, apply_v_bias)
    if key not in _cache:
        btype, tidx, tiles = _build_plan(mask_bool)
        nc = _build_program(btype, tidx, len(tiles), apply_qk_bias,
                            apply_v_bias)
        _cache[key] = (nc, tiles)
    return _cache[key]


def _core_in_map(c, xts, Wqkv, bqkv, Wo, masks_arr):
    import ml_dtypes

    BF = ml_dtypes.bfloat16
    XDT = ml_dtypes.float8_e4m3 if FP8_PROJ else BF
    # Wqk prescaled so fp8 sees ~N(0,1) weights (denormals hurt); the x32
    # per q and k is divided back out in the exp scale (SCALE/1024)
    WSCALE = 32.0 if FP8_PROJ else 1.0
    b, g = divmod(c, NCORES // B)
    hs = [HPC * g + i for i in range(HPC)]
    # wqk column chunks: [q_h0|q_h1, k_h0|k_h1, q_h2|q_h3, k_h2|k_h3]
    cols, bias_cols = [], []
    for pair in range(2):
        ha, hb = hs[2 * pair], hs[2 * pair + 1]
        for base in (0, C):  # q then k offset in Wqkv columns
            cols.append(Wqkv[:, base + ha * HD:base + (ha + 1) * HD])
            cols.append(Wqkv[:, base + hb * HD:base + (hb + 1) * HD])
            bias_cols.append(np.concatenate([
                bqkv[base + ha * HD:base + (ha + 1) * HD],
                bqkv[base + hb * HD:base + (hb + 1) * HD]]) * WSCALE)
    wqk_c = (np.concatenate(cols, axis=1) * WSCALE).astype(XDT)
    bqk_c = np.stack(bias_cols, axis=1).astype(np.float32)
    wv_c = np.concatenate(
        [Wqkv[:, 2 * C + h * HD:2 * C + (h + 1) * HD] for h in hs],
        axis=1).astype(BF)
    wo_c = np.concatenate(
        [Wo[h * HD:(h + 1) * HD, :] for h in hs], axis=0).astype(BF)
    bv_c = np.zeros((128, 2), dtype=np.float32)
    for pair in range(2):
        ha, hb = hs[2 * pair], hs[2 * pair + 1]
        bv_c[0:HD, pair] = bqkv[2 * C + ha * HD:2 * C + (ha + 1) * HD]
        bv_c[HD:128, pair] = bqkv[2 * C + hb * HD:2 * C + (hb + 1) * HD]
    m = {
        "xt": xts[b][0], "wqk": wqk_c, "wv": wv_c, "wo": wo_c,
        "masks": masks_arr, "bqk": bqk_c, "bv": bv_c,
    }
    if FP8_PROJ:
        m["xtb"] = xts[b][1]
        m["wqkb"] = (np.concatenate(cols, axis=1) * WSCALE).astype(BF)
    return m


def _prep_shared(x, tiles):
    import ml_dtypes

    BF = ml_dtypes.bfloat16
    XDT = ml_dtypes.float8_e4m3 if FP8_PROJ else BF
    PDT = ml_dtypes.float8_e4m3 if FP8_PV else BF
    # per batch: (x^T for q/k proj, x^T for v proj — bf16 copy under fp8)
    xts = []
    for b in range(B):
        xtT = np.ascontiguousarray(x[b].T)
        xts.append((xtT.astype(XDT),
                    xtT.astype(BF) if FP8_PROJ else None))
    # masks pre-doubled for the two heads sharing one exp: [n, 128, 2, 128]
    masks_arr = np.ascontiguousarray(
        np.stack([np.stack([t, t], axis=1) for t in tiles])).astype(PDT)
    return xts, masks_arr


def kernel(x, attention_mask, Wqkv, bqkv, Wo, bo, _trace=False):
    from concourse.bass_utils import run_bass_kernel_spmd

    x = np.asarray(x, dtype=np.float32)
    mask_bool = np.asarray(attention_mask)[0, 0] != 0
    Wqkv = np.asarray(Wqkv, dtype=np.float32)
    bqkv = np.asarray(bqkv, dtype=np.float32)
    Wo = np.asarray(Wo, dtype=np.float32)
    bo = np.asarray(bo, dtype=np.float32)

    apply_qk_bias = bool(np.any(bqkv[:2 * C]))
    apply_v_bias = bool(np.any(bqkv[2 * C:]))
    nc, tiles = _get_program(mask_bool, apply_qk_bias, apply_v_bias)

    xts, masks_arr = _prep_shared(x, tiles)
    in_maps = [_core_in_map(c, xts, Wqkv, bqkv, Wo, masks_arr)
               for c in range(NCORES)]

    kwargs = {}
    if _trace:
        kwargs = dict(trace=True, trace_cores=[0])
    res = run_bass_kernel_spmd(nc, in_maps, core_ids=list(range(NCORES)),
                               **kwargs)
    out = np.empty((B, T, C), dtype=np.float32)
    gpb = NCORES // B
    for b in range(B):
        acc = res.results[b * gpb]["out"].astype(np.float32)
        for g in range(1, gpb):
            acc = acc + res.results[b * gpb + g]["out"].astype(np.float32)
        out[b] = acc + bo
    if _trace:
        kernel._last_results = res
    return out
